# revision 32
# baseline (speedup 1.0000x reference)
"""BERT self-attention kernel for Trainium2, sharded over 8 NeuronCores.

Problem: nn_CustomBertSelfAttention (B=2, S=2048, D=1024, H=16 heads, HD=64).

Sharding: tensor-parallel over heads. Core c owns heads {2c, 2c+1}, i.e.
columns [128c, 128c+128) of Wq/Wk/Wv and of the output. Every core reads the
full hidden_states (transposed + cast to bf16 on the host so the contraction
dim lands on SBUF partitions with dense DMA); weights arrive host-swizzled so
their DMA is contiguous.

Per-core schedule (all matmuls bf16 with f32 PSUM accumulation):
  Phase A: projections for batch 0 (chunks 0-3 of 512 tokens): per d-tile
    Q^T/K^T/V^T accumulate in PSUM; V^T is PE-transposed per chunk into the
    per-unit [V | 1] stationary (rows pre-scaled by exp(mask), which folds
    the additive mask into the softmax exactly).
  Phase B: projections for batch 1 interleaved step-for-step with the
    scores+exp pipeline of batch-0 q-chunks 0-1, so ScalarE starts its
    128-exp workload ~30us earlier. Scores for both heads run concurrently
    via 64x128 PE row tiling (contraction is HD=64, so each head uses half
    the array); exp covers the combined [128, 1024] PSUM tile with
    scale=1/sqrt(HD) folded in (no max-subtraction - scores are O(5)).
  Phase C: remaining chunks' scores+exp steps with ctx matmuls drained from
    a queue (1-2 per step): ctx^T[65, q] += [V|1]^T @ P^T accumulated over
    k-tiles, one chain per head; row 64 is the softmax denominator.
  Outputs (unnormalized ctx + denominator row) go to the host, which divides
  during the gather (flash-attention style).
"""
import sys

sys.path.insert(0, "/opt/trn_rl_repo")

from collections import deque

import numpy as np
import ml_dtypes

from concourse import bacc
import concourse.mybir as mybir
from concourse.tile import TileContext
from concourse.masks import make_identity
from concourse.bass_utils import run_bass_kernel_spmd

B, S, D, H, HD = 2, 2048, 1024, 16, 64
N_CORES = 8
HPC = H // N_CORES          # heads per core = 2
DC = D // N_CORES           # output/weight columns per core = 128
BS = B * S                  # 4096
NU = B * HPC                # attention units per core = 4
P = 128
F32 = mybir.dt.float32
BF16 = mybir.dt.bfloat16
KT = S // P                 # 16 k-tiles per unit
ONESW = HD + 1              # V_aug width (V columns + ones column)
QH = 512                    # q chunk
NQH = S // QH               # 4 q-chunks per batch
CH = B * NQH                # 8 chunks total
LAG = 3                     # ctx trails scores by LAG steps

_cached_nc = None


def build_nc():
    nc = bacc.Bacc(None, target_bir_lowering=False)

    xT = nc.dram_tensor("xT", [D, BS], BF16, kind="ExternalInput")
    # weights arrive pre-swizzled [128, (D/128)*DC] so the DMA is contiguous
    w_in = {
        pr: nc.dram_tensor(f"w{pr}", [P, (D // P) * DC], BF16, kind="ExternalInput")
        for pr in "qkv"
    }
    bqkv = nc.dram_tensor("bqkv", [DC, 3], F32, kind="ExternalInput")
    # host-precomputed exp(mask), laid out [128, B*KT] so the DMA is contiguous
    emh = nc.dram_tensor("emh", [P, B * KT], F32, kind="ExternalInput")
    out = nc.dram_tensor("out", [NU, ONESW, S], F32, kind="ExternalOutput")

    from contextlib import ExitStack

    with TileContext(nc) as tc, ExitStack() as es:
        const = es.enter_context(tc.tile_pool(name="const", bufs=1))
        qkvp = es.enter_context(tc.tile_pool(name="qkv", bufs=1))
        wp = es.enter_context(tc.tile_pool(name="wsb", bufs=1))
        # attention pools that must coexist with the projection pools
        sp = es.enter_context(tc.tile_pool(name="sps", bufs=2, space="PSUM"))
        ptp = es.enter_context(tc.tile_pool(name="pt", bufs=3))
        obp = es.enter_context(tc.tile_pool(name="ob", bufs=2))

        ident = const.tile([P, P], BF16)
        make_identity(nc, ident)
        b_sb = const.tile([DC, 3], F32)
        nc.sync.dma_start(b_sb[:], bqkv[:])
        # exp(mask) from the host, key dim on partitions: em[p, 16*b + t]
        em = const.tile([P, B * KT], F32)
        nc.gpsimd.dma_start(em[:], emh[:])
        # dummy exp to pull the ACT table load off the critical path
        emw = const.tile([1, 1], F32)
        nc.scalar.activation(emw[:], em[0:1, 0:1], mybir.ActivationFunctionType.Exp)

        # Persistent per-core activations
        q_sb = qkvp.tile([P, BS], BF16)       # Q^T: [dq, (b s)]
        k_sb = qkvp.tile([P, BS], BF16)       # K^T
        v_aug = [
            qkvp.tile([P, KT * ONESW], BF16, tag=f"vaug{u}", name=f"vaug{u}")
            for u in range(NU)
        ]

        # Weights: w_sb[pr][:, dt*DC:(dt+1)*DC] is the d-tile dt of W slice
        w_sb = {}
        for pr in "qkv":
            w_sb[pr] = wp.tile([P, (D // P) * DC], BF16, tag=f"w{pr}", name=f"w{pr}sb")
            nc.sync.dma_start(w_sb[pr][:], w_in[pr][:])

        # ---------------- attention step emitters ----------------
        pt_t, cps_t = {}, {}
        cpp = [None]  # the cps pool, opened after the projection pools close

        def scores_step(c, kt):
            b, qh = divmod(c, NQH)
            bs0 = b * S
            q0 = bs0 + qh * QH
            k0 = bs0 + kt * P
            if kt == 0:
                pt_t[c] = ptp.tile([P, KT * 2 * QH], BF16, tag="pt", name="pt")
            sps = sp.tile([P, 2 * QH], F32, tag="sps", name="sps")
            # 64x64 PE tiling: 4 quadrant matmuls (2 heads x 2 k-halves)
            # run concurrently — row tiling alone shares one moving-operand
            # XBUS (streams serialize); 2x column tiling adds parallel
            # streams, so the whole [128, 2*QH] scores tile takes ~1 stream.
            for hl in range(HPC):
                for cj in range(2):
                    nc.tensor.matmul(
                        sps[cj * HD:(cj + 1) * HD, hl * QH:(hl + 1) * QH],
                        lhsT=k_sb[hl * HD:(hl + 1) * HD,
                                  k0 + cj * HD:k0 + (cj + 1) * HD],
                        rhs=q_sb[hl * HD:(hl + 1) * HD, q0:q0 + QH],
                        start=True, stop=True,
                        tile_position=(hl * HD, cj * HD),
                    )
            nc.scalar.activation(
                pt_t[c][:, kt * 2 * QH:(kt + 1) * 2 * QH], sps[:],
                mybir.ActivationFunctionType.Exp,
                scale=float(1.0 / np.sqrt(HD)),
            )

        def ctx_step(c, kt):
            b, qh = divmod(c, NQH)
            u0, u1 = b * HPC, b * HPC + 1
            if kt == 0:
                cps_t[c] = cpp[0].tile(
                    [ONESW, 2 * QH], F32, tag="cps", name="cps"
                )
            for hl, u in ((0, u0), (1, u1)):
                nc.tensor.matmul(
                    cps_t[c][:, hl * QH:(hl + 1) * QH],
                    lhsT=v_aug[u][:, kt * ONESW:(kt + 1) * ONESW],
                    rhs=pt_t[c][:, kt * 2 * QH + hl * QH:
                                kt * 2 * QH + (hl + 1) * QH],
                    start=(kt == 0),
                    stop=(kt == KT - 1),
                )
            if kt == KT - 1:
                # unnormalized ctx + denominator row; split per head so the
                # first DMA overlaps the second copy, separate queues.
                o = obp.tile([ONESW, 2 * QH], F32, tag="o", name="o")
                nc.vector.tensor_copy(o[:, 0:QH], cps_t[c][:, 0:QH])
                nc.sync.dma_start(
                    out[u0, :, qh * QH:(qh + 1) * QH], o[:, 0:QH]
                )
                nc.vector.tensor_copy(o[:, QH:2 * QH], cps_t[c][:, QH:2 * QH])
                nc.gpsimd.dma_start(
                    out[u1, :, qh * QH:(qh + 1) * QH], o[:, QH:2 * QH]
                )
                del cps_t[c], pt_t[c]

        # ---------------- Phases A+B: projections ----------------
        SCH = 512
        NCH = BS // SCH  # 8 projection chunks; 0-3 = batch 0, 4-7 = batch 1
        with nc.named_scope("proj"):
            with tc.tile_pool(name="xp", bufs=8) as xp, \
                 tc.tile_pool(name="vt", bufs=1) as vtp, \
                 tc.tile_pool(name="projps", bufs=1, space="PSUM") as pp, \
                 tc.tile_pool(name="tps", bufs=1, space="PSUM") as tpp:
                v_t = vtp.tile([P, BS], BF16)  # V^T staging
                ps_cur = [None]

                def proj_dt(sc, dt, engines):
                    if dt == 0:
                        ps_cur[0] = {
                            pr: pp.tile([P, SCH], F32, tag=f"ps{pr}",
                                        name=f"ps{pr}")
                            for pr in "qkv"
                        }
                    xt = xp.tile([P, SCH], BF16, tag="xt", name="xt")
                    eng = engines[(sc * (D // P) + dt) % len(engines)]
                    eng.dma_start(
                        xt[:], xT[dt * P:(dt + 1) * P, sc * SCH:(sc + 1) * SCH]
                    )
                    for pr in "qkv":
                        nc.tensor.matmul(
                            ps_cur[0][pr][:],
                            lhsT=w_sb[pr][:, dt * DC:(dt + 1) * DC],
                            rhs=xt[:],
                            start=(dt == 0),
                            stop=(dt == D // P - 1),
                        )

                def proj_finish(sc):
                    sl = slice(sc * SCH, (sc + 1) * SCH)
                    ps = ps_cur[0]
                    nc.vector.tensor_scalar_add(v_t[:, sl], ps["v"][:],
                                                b_sb[:, 2:3])
                    nc.vector.tensor_scalar_add(q_sb[:, sl], ps["q"][:],
                                                b_sb[:, 0:1])
                    nc.vector.tensor_scalar_add(k_sb[:, sl], ps["k"][:],
                                                b_sb[:, 1:2])
                    # V^T -> V for this chunk, mask-scaled into v_aug
                    for j in range(SCH // P):
                        st = sc * (SCH // P) + j
                        b, kt = divmod(st, KT)
                        tp = tpp.tile([P, P], BF16, tag="tp", name="tp")
                        nc.tensor.transpose(
                            tp[:], v_t[:, st * P:(st + 1) * P], ident[:]
                        )
                        for hl in range(HPC):
                            u = b * HPC + hl
                            nc.vector.tensor_scalar_mul(
                                v_aug[u][:, kt * ONESW:kt * ONESW + HD],
                                tp[:, hl * HD:(hl + 1) * HD],
                                em[:, st:st + 1],
                            )

                # Phase A: batch-0 projections, 3 DMA queues
                engsA = [nc.scalar, nc.sync, nc.gpsimd]
                for sc in range(NCH // 2):
                    for dt in range(D // P):
                        proj_dt(sc, dt, engsA)
                    proj_finish(sc)
                # Phase B: batch-1 projections interleaved with batch-0
                # scores+exp (ScalarE's queue is excluded from DMA duty here
                # since it now runs the exp stream).
                engsB = [nc.sync, nc.gpsimd]
                items = [(sc, dt) for sc in range(NCH // 2, NCH)
                         for dt in range(D // P)]
                for g in range(2 * KT):
                    scores_step(g // KT, g % KT)
                    sc, dt = items[g]
                    proj_dt(sc, dt, engsB)
                    if dt == D // P - 1:
                        proj_finish(sc)
                for u in range(NU):
                    b = u // HPC
                    # ones columns = exp(mask) directly
                    dst = v_aug[u][:].rearrange("p (t w) -> p t w", w=ONESW)
                    nc.vector.tensor_copy(
                        dst[:, :, HD:HD + 1].squeeze(-1),
                        em[:, b * KT:(b + 1) * KT],
                    )

        # ---------------- Phase C: attention ----------------
        with nc.named_scope("attn"):
            with tc.tile_pool(name="cps", bufs=2, space="PSUM") as cp:
                cpp[0] = cp
                # ctx work queue: chunks 0-1 (exps emitted in phase B) are
                # ready; chunks 2-7 arrive LAG steps behind their scores.
                ctxq = deque((c, kt) for c in (0, 1) for kt in range(KT))
                pending = deque()
                for g in range((CH - 2) * KT):
                    c, kt = 2 + g // KT, g % KT
                    scores_step(c, kt)
                    pending.append((c, kt))
                    if len(pending) > LAG:
                        ctxq.append(pending.popleft())
                    if ctxq:
                        ctx_step(*ctxq.popleft())
                    if g % 3 == 2 and ctxq:
                        ctx_step(*ctxq.popleft())
                while pending:
                    ctxq.append(pending.popleft())
                while ctxq:
                    ctx_step(*ctxq.popleft())

    nc.compile()
    return nc


def _prep_in_maps(hidden_states, attention_mask, Wq, bq, Wk, bk, Wv, bv):
    bf = ml_dtypes.bfloat16
    hs = np.asarray(hidden_states, dtype=np.float32).reshape(BS, D)
    xT = np.ascontiguousarray(hs.T).astype(bf)
    # em[p, b*KT + t] = exp(mask[b, t*128 + p])
    em = np.exp(np.asarray(attention_mask, dtype=np.float32).reshape(B, S))
    emh = np.ascontiguousarray(
        em.reshape(B, KT, P).transpose(2, 0, 1).reshape(P, B * KT)
    )
    Ws = {"q": np.asarray(Wq, np.float32), "k": np.asarray(Wk, np.float32),
          "v": np.asarray(Wv, np.float32)}
    bs = {"q": np.asarray(bq, np.float32), "k": np.asarray(bk, np.float32),
          "v": np.asarray(bv, np.float32)}
    in_maps = []
    for c in range(N_CORES):
        sl = slice(c * DC, (c + 1) * DC)
        m = {"xT": xT, "emh": emh}
        for pr in "qkv":
            # pre-swizzle [D, DC] -> [128, (D/128)*DC]: w[p, t*DC+n] = W[t*128+p, n]
            wsl = Ws[pr][:, sl].reshape(D // P, P, DC).transpose(1, 0, 2)
            m[f"w{pr}"] = np.ascontiguousarray(wsl.reshape(P, -1)).astype(bf)
        m["bqkv"] = np.ascontiguousarray(
            np.stack([bs["q"][sl], bs["k"][sl], bs["v"][sl]], axis=1)
        )
        in_maps.append(m)
    return in_maps


def _gather(results):
    full = np.empty((B, S, D), dtype=np.float32)
    for c in range(N_CORES):
        o = results[c]["out"]  # [NU, ONESW, S]: rows 0..63 ctx, row 64 denom
        for b in range(B):
            for hl in range(HPC):
                col = c * DC + hl * HD
                u = b * HPC + hl
                full[b, :, col:col + HD] = (o[u, :HD] / o[u, HD:HD + 1]).T
    return full


def kernel(hidden_states, attention_mask, Wq, bq, Wk, bk, Wv, bv, **run_kwargs):
    global _cached_nc
    if _cached_nc is None:
        _cached_nc = build_nc()
    in_maps = _prep_in_maps(
        hidden_states, attention_mask, Wq, bq, Wk, bk, Wv, bv
    )
    res = run_bass_kernel_spmd(
        _cached_nc, in_maps, core_ids=list(range(N_CORES)), **run_kwargs
    )
    full = _gather(res.results)
    if run_kwargs:
        kernel.last_result = res
    return full


# revision 33
# speedup vs baseline: 1.0324x; 1.0324x over previous
"""BERT self-attention kernel for Trainium2, sharded over 8 NeuronCores.

Problem: nn_CustomBertSelfAttention (B=2, S=2048, D=1024, H=16 heads, HD=64).

Sharding: tensor-parallel over heads. Core c owns heads {2c, 2c+1}, i.e.
columns [128c, 128c+128) of Wq/Wk/Wv and of the output. Every core reads the
full hidden_states (transposed + cast to bf16 on the host so the contraction
dim lands on SBUF partitions with dense DMA); weights arrive host-swizzled so
their DMA is contiguous.

Per-core schedule (all matmuls bf16 with f32 PSUM accumulation):
  Phase A: projections for batch 0 (chunks 0-3 of 512 tokens): per d-tile
    Q^T/K^T/V^T accumulate in PSUM; V^T is PE-transposed per chunk into the
    per-unit [V | 1] stationary (rows pre-scaled by exp(mask), which folds
    the additive mask into the softmax exactly).
  Phase B: projections for batch 1 interleaved step-for-step with the
    scores+exp pipeline of batch-0 q-chunks 0-1, so ScalarE starts its
    128-exp workload ~30us earlier. Scores for both heads run concurrently
    via 64x128 PE row tiling (contraction is HD=64, so each head uses half
    the array); exp covers the combined [128, 1024] PSUM tile with
    scale=1/sqrt(HD) folded in (no max-subtraction - scores are O(5)).
  Phase C: remaining chunks' scores+exp steps with ctx matmuls drained from
    a queue (1-2 per step): ctx^T[65, q] += [V|1]^T @ P^T accumulated over
    k-tiles, one chain per head; row 64 is the softmax denominator.
  Outputs (unnormalized ctx + denominator row) go to the host, which divides
  during the gather (flash-attention style).
"""
import sys

sys.path.insert(0, "/opt/trn_rl_repo")

from collections import deque

import numpy as np
import ml_dtypes

from concourse import bacc
import concourse.mybir as mybir
from concourse.tile import TileContext
from concourse.masks import make_identity
from concourse.bass_utils import run_bass_kernel_spmd

B, S, D, H, HD = 2, 2048, 1024, 16, 64
N_CORES = 8
HPC = H // N_CORES          # heads per core = 2
DC = D // N_CORES           # output/weight columns per core = 128
BS = B * S                  # 4096
NU = B * HPC                # attention units per core = 4
P = 128
F32 = mybir.dt.float32
BF16 = mybir.dt.bfloat16
KT = S // P                 # 16 k-tiles per unit
ONESW = HD + 1              # V_aug width (V columns + ones column)
QH = 512                    # q chunk
NQH = S // QH               # 4 q-chunks per batch
CH = B * NQH                # 8 chunks total
LAG = 3                     # ctx trails scores by LAG steps

_cached_nc = None


def build_nc():
    nc = bacc.Bacc(None, target_bir_lowering=False)

    xT = nc.dram_tensor("xT", [D, BS], BF16, kind="ExternalInput")
    # weights arrive pre-swizzled [128, (D/128)*DC] so the DMA is contiguous
    w_in = {
        pr: nc.dram_tensor(f"w{pr}", [P, (D // P) * DC], BF16, kind="ExternalInput")
        for pr in "qkv"
    }
    bqkv = nc.dram_tensor("bqkv", [DC, 3], F32, kind="ExternalInput")
    # host-precomputed exp(mask), laid out [128, B*KT] so the DMA is contiguous
    emh = nc.dram_tensor("emh", [P, B * KT], F32, kind="ExternalInput")
    out = nc.dram_tensor("out", [NU, ONESW, S], F32, kind="ExternalOutput")

    from contextlib import ExitStack

    with TileContext(nc) as tc, ExitStack() as es:
        const = es.enter_context(tc.tile_pool(name="const", bufs=1))
        qkvp = es.enter_context(tc.tile_pool(name="qkv", bufs=1))
        wp = es.enter_context(tc.tile_pool(name="wsb", bufs=1))
        # attention pools that must coexist with the projection pools
        sp = es.enter_context(tc.tile_pool(name="sps", bufs=2, space="PSUM"))
        ptp = es.enter_context(tc.tile_pool(name="pt", bufs=3))
        obp = es.enter_context(tc.tile_pool(name="ob", bufs=2))

        ident = const.tile([P, P], BF16)
        make_identity(nc, ident)
        b_sb = const.tile([DC, 3], F32)
        nc.sync.dma_start(b_sb[:], bqkv[:])
        # exp(mask) from the host, key dim on partitions: em[p, 16*b + t]
        em = const.tile([P, B * KT], F32)
        nc.gpsimd.dma_start(em[:], emh[:])
        # dummy exp to pull the ACT table load off the critical path
        emw = const.tile([1, 1], F32)
        nc.scalar.activation(emw[:], em[0:1, 0:1], mybir.ActivationFunctionType.Exp)

        # Persistent per-core activations
        q_sb = qkvp.tile([P, BS], BF16)       # Q^T: [dq, (b s)]
        k_sb = qkvp.tile([P, BS], BF16)       # K^T
        v_aug = [
            qkvp.tile([P, KT * ONESW], BF16, tag=f"vaug{u}", name=f"vaug{u}")
            for u in range(NU)
        ]

        # Weights: w_sb[pr][:, dt*DC:(dt+1)*DC] is the d-tile dt of W slice
        w_sb = {}
        for pr in "qkv":
            w_sb[pr] = wp.tile([P, (D // P) * DC], BF16, tag=f"w{pr}", name=f"w{pr}sb")
            nc.sync.dma_start(w_sb[pr][:], w_in[pr][:])

        # ---------------- attention step emitters ----------------
        pt_t, cps_t = {}, {}
        cpp = [None]  # the cps pool, opened after the projection pools close

        def scores_step(c, kt):
            b, qh = divmod(c, NQH)
            bs0 = b * S
            q0 = bs0 + qh * QH
            k0 = bs0 + kt * P
            if kt == 0:
                pt_t[c] = ptp.tile([P, KT * 2 * QH], BF16, tag="pt", name="pt")
            sps = sp.tile([P, 2 * QH], F32, tag="sps", name="sps")
            # both heads via 64x128 PE row tiling (the two streams share the
            # moving-operand XBUS, but stationaries coexist so no LDW stalls;
            # measured faster than 64x64 quad-tiling, which adds dispatch
            # overhead without delivering parallel streams)
            nc.tensor.matmul(
                sps[:, 0:QH],
                lhsT=k_sb[0:HD, k0:k0 + P],
                rhs=q_sb[0:HD, q0:q0 + QH],
                start=True, stop=True,
                tile_position=(0, 0),
            )
            nc.tensor.matmul(
                sps[:, QH:2 * QH],
                lhsT=k_sb[HD:P, k0:k0 + P],
                rhs=q_sb[HD:P, q0:q0 + QH],
                start=True, stop=True,
                tile_position=(64, 0),
            )
            nc.scalar.activation(
                pt_t[c][:, kt * 2 * QH:(kt + 1) * 2 * QH], sps[:],
                mybir.ActivationFunctionType.Exp,
                scale=float(1.0 / np.sqrt(HD)),
            )

        def ctx_step(c, kt):
            b, qh = divmod(c, NQH)
            u0, u1 = b * HPC, b * HPC + 1
            if kt == 0:
                cps_t[c] = cpp[0].tile(
                    [ONESW, 2 * QH], F32, tag="cps", name="cps"
                )
            for hl, u in ((0, u0), (1, u1)):
                nc.tensor.matmul(
                    cps_t[c][:, hl * QH:(hl + 1) * QH],
                    lhsT=v_aug[u][:, kt * ONESW:(kt + 1) * ONESW],
                    rhs=pt_t[c][:, kt * 2 * QH + hl * QH:
                                kt * 2 * QH + (hl + 1) * QH],
                    start=(kt == 0),
                    stop=(kt == KT - 1),
                )
            if kt == KT - 1:
                # unnormalized ctx + denominator row; split per head so the
                # first DMA overlaps the second copy, separate queues.
                o = obp.tile([ONESW, 2 * QH], F32, tag="o", name="o")
                nc.vector.tensor_copy(o[:, 0:QH], cps_t[c][:, 0:QH])
                nc.sync.dma_start(
                    out[u0, :, qh * QH:(qh + 1) * QH], o[:, 0:QH]
                )
                nc.vector.tensor_copy(o[:, QH:2 * QH], cps_t[c][:, QH:2 * QH])
                nc.gpsimd.dma_start(
                    out[u1, :, qh * QH:(qh + 1) * QH], o[:, QH:2 * QH]
                )
                del cps_t[c], pt_t[c]

        # ---------------- Phases A+B: projections ----------------
        SCH = 512
        NCH = BS // SCH  # 8 projection chunks; 0-3 = batch 0, 4-7 = batch 1
        with nc.named_scope("proj"):
            with tc.tile_pool(name="xp", bufs=8) as xp, \
                 tc.tile_pool(name="vt", bufs=1) as vtp, \
                 tc.tile_pool(name="projps", bufs=1, space="PSUM") as pp, \
                 tc.tile_pool(name="tps", bufs=1, space="PSUM") as tpp:
                v_t = vtp.tile([P, BS], BF16)  # V^T staging
                ps_cur = [None]

                def proj_dt(sc, dt, engines):
                    if dt == 0:
                        ps_cur[0] = {
                            pr: pp.tile([P, SCH], F32, tag=f"ps{pr}",
                                        name=f"ps{pr}")
                            for pr in "qkv"
                        }
                    xt = xp.tile([P, SCH], BF16, tag="xt", name="xt")
                    eng = engines[(sc * (D // P) + dt) % len(engines)]
                    eng.dma_start(
                        xt[:], xT[dt * P:(dt + 1) * P, sc * SCH:(sc + 1) * SCH]
                    )
                    for pr in "qkv":
                        nc.tensor.matmul(
                            ps_cur[0][pr][:],
                            lhsT=w_sb[pr][:, dt * DC:(dt + 1) * DC],
                            rhs=xt[:],
                            start=(dt == 0),
                            stop=(dt == D // P - 1),
                        )

                def proj_finish(sc):
                    sl = slice(sc * SCH, (sc + 1) * SCH)
                    ps = ps_cur[0]
                    nc.vector.tensor_scalar_add(v_t[:, sl], ps["v"][:],
                                                b_sb[:, 2:3])
                    nc.vector.tensor_scalar_add(q_sb[:, sl], ps["q"][:],
                                                b_sb[:, 0:1])
                    nc.vector.tensor_scalar_add(k_sb[:, sl], ps["k"][:],
                                                b_sb[:, 1:2])
                    # V^T -> V for this chunk, mask-scaled into v_aug
                    for j in range(SCH // P):
                        st = sc * (SCH // P) + j
                        b, kt = divmod(st, KT)
                        tp = tpp.tile([P, P], BF16, tag="tp", name="tp")
                        nc.tensor.transpose(
                            tp[:], v_t[:, st * P:(st + 1) * P], ident[:]
                        )
                        for hl in range(HPC):
                            u = b * HPC + hl
                            nc.vector.tensor_scalar_mul(
                                v_aug[u][:, kt * ONESW:kt * ONESW + HD],
                                tp[:, hl * HD:(hl + 1) * HD],
                                em[:, st:st + 1],
                            )

                # Phase A: batch-0 projections, 3 DMA queues
                engsA = [nc.scalar, nc.sync, nc.gpsimd]
                for sc in range(NCH // 2):
                    for dt in range(D // P):
                        proj_dt(sc, dt, engsA)
                    proj_finish(sc)
                # Phase B: batch-1 projections interleaved with batch-0
                # scores+exp (ScalarE's queue is excluded from DMA duty here
                # since it now runs the exp stream).
                engsB = [nc.sync, nc.gpsimd]
                items = [(sc, dt) for sc in range(NCH // 2, NCH)
                         for dt in range(D // P)]
                for g in range(2 * KT):
                    scores_step(g // KT, g % KT)
                    sc, dt = items[g]
                    proj_dt(sc, dt, engsB)
                    if dt == D // P - 1:
                        proj_finish(sc)
                for u in range(NU):
                    b = u // HPC
                    # ones columns = exp(mask) directly
                    dst = v_aug[u][:].rearrange("p (t w) -> p t w", w=ONESW)
                    nc.vector.tensor_copy(
                        dst[:, :, HD:HD + 1].squeeze(-1),
                        em[:, b * KT:(b + 1) * KT],
                    )

        # ---------------- Phase C: attention ----------------
        with nc.named_scope("attn"):
            with tc.tile_pool(name="cps", bufs=2, space="PSUM") as cp:
                cpp[0] = cp
                # ctx work queue: chunks 0-1 (exps emitted in phase B) are
                # ready; chunks 2-7 arrive LAG steps behind their scores.
                ctxq = deque((c, kt) for c in (0, 1) for kt in range(KT))
                pending = deque()
                for g in range((CH - 2) * KT):
                    c, kt = 2 + g // KT, g % KT
                    scores_step(c, kt)
                    pending.append((c, kt))
                    if len(pending) > LAG:
                        ctxq.append(pending.popleft())
                    if ctxq:
                        ctx_step(*ctxq.popleft())
                    if g % 3 == 2 and ctxq:
                        ctx_step(*ctxq.popleft())
                while pending:
                    ctxq.append(pending.popleft())
                while ctxq:
                    ctx_step(*ctxq.popleft())

    nc.compile()
    return nc


def _prep_in_maps(hidden_states, attention_mask, Wq, bq, Wk, bk, Wv, bv):
    bf = ml_dtypes.bfloat16
    hs = np.asarray(hidden_states, dtype=np.float32).reshape(BS, D)
    xT = np.ascontiguousarray(hs.T).astype(bf)
    # em[p, b*KT + t] = exp(mask[b, t*128 + p])
    em = np.exp(np.asarray(attention_mask, dtype=np.float32).reshape(B, S))
    emh = np.ascontiguousarray(
        em.reshape(B, KT, P).transpose(2, 0, 1).reshape(P, B * KT)
    )
    Ws = {"q": np.asarray(Wq, np.float32), "k": np.asarray(Wk, np.float32),
          "v": np.asarray(Wv, np.float32)}
    bs = {"q": np.asarray(bq, np.float32), "k": np.asarray(bk, np.float32),
          "v": np.asarray(bv, np.float32)}
    in_maps = []
    for c in range(N_CORES):
        sl = slice(c * DC, (c + 1) * DC)
        m = {"xT": xT, "emh": emh}
        for pr in "qkv":
            # pre-swizzle [D, DC] -> [128, (D/128)*DC]: w[p, t*DC+n] = W[t*128+p, n]
            wsl = Ws[pr][:, sl].reshape(D // P, P, DC).transpose(1, 0, 2)
            m[f"w{pr}"] = np.ascontiguousarray(wsl.reshape(P, -1)).astype(bf)
        m["bqkv"] = np.ascontiguousarray(
            np.stack([bs["q"][sl], bs["k"][sl], bs["v"][sl]], axis=1)
        )
        in_maps.append(m)
    return in_maps


def _gather(results):
    full = np.empty((B, S, D), dtype=np.float32)
    for c in range(N_CORES):
        o = results[c]["out"]  # [NU, ONESW, S]: rows 0..63 ctx, row 64 denom
        for b in range(B):
            for hl in range(HPC):
                col = c * DC + hl * HD
                u = b * HPC + hl
                full[b, :, col:col + HD] = (o[u, :HD] / o[u, HD:HD + 1]).T
    return full


def kernel(hidden_states, attention_mask, Wq, bq, Wk, bk, Wv, bv, **run_kwargs):
    global _cached_nc
    if _cached_nc is None:
        _cached_nc = build_nc()
    in_maps = _prep_in_maps(
        hidden_states, attention_mask, Wq, bq, Wk, bk, Wv, bv
    )
    res = run_bass_kernel_spmd(
        _cached_nc, in_maps, core_ids=list(range(N_CORES)), **run_kwargs
    )
    full = _gather(res.results)
    if run_kwargs:
        kernel.last_result = res
    return full
